# revision 1
# baseline (speedup 1.0000x reference)
"""ArcFace loss on 8 TRN2 NeuronCores, sharded along the class dim C.

Per core: stream the [512, 12500] cosine shard through ScalarE
exp(64*x - 64) with per-row accum_out row-sums (a single HBM pass at
~330 GB/s/core), gather each row's target element via indirect DMA,
apply the angular margin on tiny [128,4] tensors, AllGather the
per-row (sumexp, target-logit) partials across the 8 cores, reduce
locally, then loss = 64 + mean(log(sumexp) - tgt).

Since cosine <= 1, logits are <= 64, so exp(S*c - 64) <= 1 and the
usual max-pass of logsumexp is unnecessary: logZ = 64 + log(sum).
The margin term is folded in by correcting the target column's
exp contribution (own * (exp(S*phi-64) - exp(S*c-64))).
"""

import math
import os

import numpy as np

import concourse.bacc as bacc
import concourse.bass as bass
import concourse.bass_isa as bass_isa
import concourse.mybir as mybir
import concourse.tile as tile
from concourse.bass_utils import run_bass_kernel_spmd

# ArcFace constants (match the reference)
S = 64.0
M = 0.5
COS_M = math.cos(M)
SIN_M = math.sin(M)
TH = math.cos(math.pi - M)
MM = math.sin(math.pi - M) * M
EPS = 1e-07

B, C = 512, 100000
NCORES = 8
CS = C // NCORES  # 12500 classes per core
P = 128
RT = B // P  # 4 row tiles
FC = int(os.environ.get("K_FC", "3125"))  # steady-state chunk width
RAMP_PLAN = [FC] * (CS // FC)
TAIL_PLAN = [FC] * (CS // FC)
NCH = CS // FC
SHIFT = 64.0  # exp(S*c - SHIFT) keeps everything <= 1 since c in [-1, 1]

F32 = mybir.dt.float32
I32 = mybir.dt.int32


CC_KIND = os.environ.get("K_CC", "ag")
DUAL_RING = os.environ.get("K_DUAL", "0") == "1"


def _patch_act_tables():
    """Make natural_log_exp_and_others the only provider of Exp/Ln so the
    table-load pass emits a single ACT_TABLE_LOAD instead of thrashing
    between the exp-only and ln-only sets. Set ids stay file-ordered."""
    import concourse.hw_specs as hw_specs

    orig = hw_specs.get_activation_tables
    if getattr(orig, "_arcface_patched", False):
        return

    def patched(arch):
        tabs = {k: set(v) for k, v in orig(arch).items()}
        for name, fns in tabs.items():
            if name != "natural_log_exp_and_others":
                fns.discard(mybir.ActivationFunctionType.Exp)
                fns.discard(mybir.ActivationFunctionType.Ln)
        return tabs

    patched._arcface_patched = True
    hw_specs.get_activation_tables = patched
    bacc.get_activation_tables = patched


def build_nc():
    _patch_act_tables()
    nc = bacc.Bacc(None)
    cos_p = nc.declare_dram_parameter("cosine", [B, CS], F32, isOutput=False)
    gidx_p = nc.declare_dram_parameter("gidx", [P, RT], I32, isOutput=False)
    own_p = nc.declare_dram_parameter("own", [P, RT], F32, isOutput=False)
    out_p = nc.declare_dram_parameter("out", [1, 1], F32, isOutput=True)

    cos_flat = cos_p[:].rearrange("b (c o) -> (b c) o", o=1)

    with tile.TileContext(nc) as tc:
        with (
            tc.tile_pool(name="data", bufs=int(os.environ.get("K_BUFS", "3"))) as data_pool,
            tc.tile_pool(name="expp", bufs=2) as exp_pool,
            tc.tile_pool(name="small", bufs=1) as small,
            tc.tile_pool(name="dram", bufs=1, space="DRAM") as dram,
        ):
            # bias operand for exp(S*x - SHIFT) activations
            nbias = small.tile([P, 1], F32)
            nc.gpsimd.memset(nbias[:], -SHIFT)
            # dummy activation: pulls the ACT table load to the start of the
            # kernel instead of gating the first streaming exp
            warm_act = small.tile([P, 1], F32)
            nc.scalar.activation(
                out=warm_act[:], in_=nbias[:], func=mybir.ActivationFunctionType.Exp
            )

            # ---- main streaming pass: exp + row-sum accumulate
            plans = [RAMP_PLAN] + [[FC] * NCH] * (RT - 2) + [TAIL_PLAN]
            assert all(sum(p) == CS for p in plans)
            fc_max = max(max(p) for p in plans)
            tcols = []  # per row tile: (start, count) in sums
            ncols = 0
            for t in range(RT):
                tcols.append((ncols, len(plans[t])))
                ncols += len(plans[t])
            sums = small.tile([P, ncols], F32)
            for t in range(RT):
                off = 0
                for i, w in enumerate(plans[t]):
                    dt = data_pool.tile([P, fc_max], F32, tag="data")
                    dma_eng = (
                        nc.scalar
                        if DUAL_RING and (t * len(plans[t]) + i) % 2 == 1
                        else nc.sync
                    )
                    dma_eng.dma_start(
                        out=dt[:, 0:w],
                        in_=cos_p[t * P : (t + 1) * P, off : off + w],
                    )
                    ev = exp_pool.tile([P, fc_max], F32, tag="exp")
                    col = tcols[t][0] + i
                    nc.scalar.activation(
                        out=ev[:, 0:w], in_=dt[:, 0:w],
                        func=mybir.ActivationFunctionType.Exp,
                        scale=S, bias=nbias[:],
                        accum_out=sums[:, col : col + 1],
                    )
                    off += w

            # ---- gather target elements: idx in SBUF -> indirect DMA
            idx_sb = small.tile([P, RT], I32)
            own_sb = small.tile([P, RT], F32)
            gc = small.tile([P, RT], F32)  # gathered cosine at target cols
            nc.gpsimd.dma_start(out=idx_sb[:], in_=gidx_p[:])
            nc.gpsimd.dma_start(out=own_sb[:], in_=own_p[:])
            for t in range(RT):
                nc.gpsimd.indirect_dma_start(
                    out=gc[:, t : t + 1],
                    out_offset=None,
                    in_=cos_flat,
                    in_offset=bass.IndirectOffsetOnAxis(ap=idx_sb[:, t : t + 1], axis=0),
                )

            # ---- margin math on [128, RT] tensors
            cc_sb = small.tile([P, 2 * RT], F32)  # cols 0:RT sumexp, RT:2RT tgt
            spart = cc_sb[:, 0:RT]
            tpart = cc_sb[:, RT : 2 * RT]

            c = small.tile([P, RT], F32)
            nc.vector.tensor_scalar(
                out=c[:], in0=gc[:], scalar1=1.0 - EPS, scalar2=-1.0 + EPS,
                op0=mybir.AluOpType.min, op1=mybir.AluOpType.max,
            )
            # om = 1 - c^2  (via (c*c)*-1 + 1)
            om = small.tile([P, RT], F32)
            nc.vector.tensor_tensor(out=om[:], in0=c[:], in1=c[:], op=mybir.AluOpType.mult)
            nc.vector.tensor_scalar(
                out=om[:], in0=om[:], scalar1=-1.0, scalar2=1.0,
                op0=mybir.AluOpType.mult, op1=mybir.AluOpType.add,
            )
            # sine = exp(0.5 * ln(om)) — avoids the low-precision Sqrt table
            sine = small.tile([P, RT], F32)
            nc.scalar.activation(out=sine[:], in_=om[:], func=mybir.ActivationFunctionType.Ln)
            nc.scalar.activation(
                out=sine[:], in_=sine[:], func=mybir.ActivationFunctionType.Exp, scale=0.5
            )
            # phi = c*COS_M - sine*SIN_M
            phi = small.tile([P, RT], F32)
            t1 = small.tile([P, RT], F32)
            nc.vector.tensor_scalar(out=t1[:], in0=sine[:], scalar1=SIN_M, scalar2=None, op0=mybir.AluOpType.mult)
            nc.vector.scalar_tensor_tensor(
                out=phi[:], in0=c[:], scalar=COS_M, in1=t1[:],
                op0=mybir.AluOpType.mult, op1=mybir.AluOpType.subtract,
            )
            # phi = where(c > TH, phi, c - MM)
            gt = small.tile([P, RT], F32)
            nc.vector.tensor_scalar(out=gt[:], in0=c[:], scalar1=TH, scalar2=None, op0=mybir.AluOpType.is_gt)
            cmm = small.tile([P, RT], F32)
            nc.vector.tensor_scalar(out=cmm[:], in0=c[:], scalar1=MM, scalar2=None, op0=mybir.AluOpType.subtract)
            d = small.tile([P, RT], F32)
            nc.vector.tensor_tensor(out=d[:], in0=phi[:], in1=cmm[:], op=mybir.AluOpType.subtract)
            nc.vector.tensor_tensor(out=d[:], in0=d[:], in1=gt[:], op=mybir.AluOpType.mult)
            nc.vector.tensor_tensor(out=phi[:], in0=cmm[:], in1=d[:], op=mybir.AluOpType.add)

            # tpart = own * phi * S
            nc.vector.tensor_tensor(out=tpart, in0=own_sb[:], in1=phi[:], op=mybir.AluOpType.mult)
            nc.vector.tensor_scalar(out=tpart, in0=tpart, scalar1=S, scalar2=None, op0=mybir.AluOpType.mult)

            # delta = own * (exp(S*phi - SHIFT) - exp(S*c - SHIFT))
            e_phi = small.tile([P, RT], F32)
            e_c = small.tile([P, RT], F32)
            nc.scalar.activation(
                out=e_phi[:], in_=phi[:], func=mybir.ActivationFunctionType.Exp,
                scale=S, bias=nbias[:],
            )
            nc.scalar.activation(
                out=e_c[:], in_=c[:], func=mybir.ActivationFunctionType.Exp,
                scale=S, bias=nbias[:],
            )
            delta = small.tile([P, RT], F32)
            nc.vector.tensor_tensor(out=delta[:], in0=e_phi[:], in1=e_c[:], op=mybir.AluOpType.subtract)
            nc.vector.tensor_tensor(out=delta[:], in0=delta[:], in1=own_sb[:], op=mybir.AluOpType.mult)


            # spart[:, t] = sum_k sums[:, tcols[t]] + delta[:, t]
            red = small.tile([P, RT], F32)
            for t in range(RT):
                lo, n = tcols[t]
                nc.vector.tensor_reduce(
                    out=red[:, t : t + 1],
                    in_=sums[:, lo : lo + n],
                    axis=mybir.AxisListType.X,
                    op=mybir.AluOpType.add,
                )
            nc.vector.tensor_tensor(out=spart, in0=red[:], in1=delta[:], op=mybir.AluOpType.add)

            # ---- cross-core reduction of the [128, 2*RT] stats
            cc_in = dram.tile([P, 2 * RT], F32)
            nc.sync.dma_start(out=cc_in[:], in_=cc_sb[:])
            red_sb = small.tile([P, 2 * RT], F32)
            if CC_KIND == "ag":
                # AllGather (floor ~2x cheaper than AllReduce) + local reduce
                cc_gath = dram.tile([NCORES * P, 2 * RT], F32)
                nc.gpsimd.collective_compute(
                    "AllGather",
                    mybir.AluOpType.bypass,
                    replica_groups=[list(range(NCORES))],
                    ins=[cc_in.opt()],
                    outs=[cc_gath.opt()],
                )
                gath_sb = small.tile([P, NCORES * 2 * RT], F32)
                nc.sync.dma_start(
                    out=gath_sb[:],
                    in_=cc_gath[:].rearrange("(r p) j -> p r j", p=P),
                )
                nc.vector.tensor_reduce(
                    out=red_sb[:],
                    in_=gath_sb[:].rearrange("p (r j) -> p j r", r=NCORES),
                    axis=mybir.AxisListType.X,
                    op=mybir.AluOpType.add,
                )
            else:
                cc_out = dram.tile([P, 2 * RT], F32)
                nc.gpsimd.collective_compute(
                    "AllReduce",
                    mybir.AluOpType.add,
                    replica_groups=[list(range(NCORES))],
                    ins=[cc_in.opt()],
                    outs=[cc_out.opt()],
                )
                nc.sync.dma_start(out=red_sb[:], in_=cc_out[:])

            # ---- loss = SHIFT + mean(log(sumexp) - tgt)
            logs = small.tile([P, RT], F32)
            nc.scalar.activation(
                out=logs[:], in_=red_sb[:, 0:RT], func=mybir.ActivationFunctionType.Ln
            )
            lvec = small.tile([P, RT], F32)
            nc.vector.tensor_tensor(
                out=lvec[:], in0=logs[:], in1=red_sb[:, RT : 2 * RT],
                op=mybir.AluOpType.subtract,
            )
            lrow = small.tile([P, 1], F32)
            nc.vector.tensor_reduce(
                out=lrow[:], in_=lvec[:], axis=mybir.AxisListType.X, op=mybir.AluOpType.add
            )
            ltot = small.tile([P, 1], F32)
            nc.gpsimd.partition_all_reduce(
                ltot[:], lrow[:], channels=P, reduce_op=bass_isa.ReduceOp.add
            )
            res = small.tile([1, 1], F32)
            nc.scalar.activation(
                out=res[:], in_=ltot[0:1, :], func=mybir.ActivationFunctionType.Copy,
                scale=1.0 / B, bias=SHIFT,
            )
            nc.sync.dma_start(out=out_p[:], in_=res[:])

    nc.finalize()
    return nc


_CACHE = {}


def _get_nc():
    if "nc" not in _CACHE:
        _CACHE["nc"] = build_nc()
    return _CACHE["nc"]


def make_in_maps(cosine: np.ndarray, labels: np.ndarray):
    labels = np.asarray(labels).astype(np.int64)
    rows = np.arange(B, dtype=np.int64)
    in_maps = []
    for m in range(NCORES):
        lo = m * CS
        owned = (labels >= lo) & (labels < lo + CS)
        local = np.where(owned, labels - lo, 0)
        gidx = np.ascontiguousarray((rows * CS + local).astype(np.int32).reshape(RT, P).T)
        own = np.ascontiguousarray(owned.astype(np.float32).reshape(RT, P).T)
        shard = np.ascontiguousarray(cosine[:, lo : lo + CS], dtype=np.float32)
        in_maps.append({"cosine": shard, "gidx": gidx, "own": own})
    return in_maps


def kernel(cosine: np.ndarray, labels: np.ndarray, _trace: bool = False):
    nc = _get_nc()
    in_maps = make_in_maps(np.asarray(cosine, dtype=np.float32), labels)
    res = run_bass_kernel_spmd(
        nc, in_maps, core_ids=list(range(NCORES)), trace=_trace
    )
    out = np.asarray(res.results[0]["out"], dtype=np.float32).reshape(())
    if _trace:
        return out, res
    return out



# revision 6
# speedup vs baseline: 1.4447x; 1.4447x over previous
"""ArcFace loss on 8 TRN2 NeuronCores, sharded along the batch dim B.

Each core takes 64 rows x all 100k classes (25.6 MB) so the whole loss
for those rows is computed locally — no inter-core collective at all
(the baseline's AllGather + core-skew wait cost ~25us of pure tail).

Per-core layout: the 64 rows are split into column halves so all 128
SBUF partitions stream: partition p = h*64 + r holds row r, columns
[h*50000, (h+1)*50000). ScalarE computes exp(64c - 64) with per-row
accum_out row sums in a single HBM pass (~341 GB/s measured). The
per-partition half-sums are folded back to per-row sums with a tiny
TensorE matmul against a [128, 64] pair-fold matrix, then
loss_r = log(sum_r) - 64*phi_r and a gpsimd partition reduce gives the
per-core partial. Host adds the 8 partials: loss = 64 + sum/512.

Since cosine <= 1, logits are <= 64, so exp(64c - 64) <= 1 and the
max-pass of logsumexp is unnecessary: logZ = 64 + log(sum). The margin
at the target column is folded in by correcting its exp contribution
(delta = exp(64*phi - 64) - exp(64*c - 64)).
"""

import math

import numpy as np

import concourse.bacc as bacc
import concourse.bass as bass
import concourse.bass_isa as bass_isa
import concourse.mybir as mybir
import concourse.tile as tile
from concourse.bass_utils import run_bass_kernel_spmd

# ArcFace constants (match the reference)
S = 64.0
M = 0.5
COS_M = math.cos(M)
SIN_M = math.sin(M)
TH = math.cos(math.pi - M)
MM = math.sin(math.pi - M) * M
EPS = 1e-07

B, C = 512, 100000
NCORES = 8
RPC = B // NCORES  # 64 rows per core
HALF = C // 2  # 50000 columns per partition-row
P = 128
SHIFT = 64.0  # exp(S*c - SHIFT) keeps everything <= 1 since c in [0, 1)

# Chunk plan over the 50000 streamed columns per partition. Uniform big
# chunks, tapered at the end so the final ScalarE exp (which can only
# start once the last DMA lands) is short.
CHUNKS = [3160] * 15 + [1200, 800, 600]
assert sum(CHUNKS) == HALF
NCH = len(CHUNKS)
FC_MAX = max(CHUNKS)

F32 = mybir.dt.float32
I32 = mybir.dt.int32


def _patch_act_tables():
    """Make natural_log_exp_and_others the only provider of Exp/Ln so the
    table-load pass emits a single ACT_TABLE_LOAD instead of thrashing
    between the exp-only and ln-only sets."""
    import concourse.hw_specs as hw_specs

    orig = hw_specs.get_activation_tables
    if getattr(orig, "_arcface_patched", False):
        return

    def patched(arch):
        tabs = {k: set(v) for k, v in orig(arch).items()}
        for name, fns in tabs.items():
            if name != "natural_log_exp_and_others":
                fns.discard(mybir.ActivationFunctionType.Exp)
                fns.discard(mybir.ActivationFunctionType.Ln)
        return tabs

    patched._arcface_patched = True
    hw_specs.get_activation_tables = patched
    bacc.get_activation_tables = patched


def build_nc():
    _patch_act_tables()
    nc = bacc.Bacc(None)
    cos_p = nc.declare_dram_parameter("cosine", [RPC, C], F32, isOutput=False)
    gidx_p = nc.declare_dram_parameter("gidx", [RPC, 1], I32, isOutput=False)
    out_p = nc.declare_dram_parameter("out", [1, 1], F32, isOutput=True)

    # [128, 50000]: partition 2r+h = row r, column half h
    cos_r = cos_p[:].rearrange("r (h c) -> (r h) c", h=2)
    cos_flat = cos_p[:].rearrange("r (c o) -> (r c) o", o=1)

    with tile.TileContext(nc) as tc:
        with (
            tc.tile_pool(name="data", bufs=6) as data_pool,
            tc.tile_pool(name="expp", bufs=2) as exp_pool,
            tc.tile_pool(name="small", bufs=1) as small,
            tc.tile_pool(name="psum", bufs=1, space="PSUM") as psum,
        ):
            # bias operand for exp(S*x - SHIFT) activations
            nbias = small.tile([P, 1], F32)
            nc.gpsimd.memset(nbias[:], -SHIFT)
            # dummy activation: pulls the ACT table load to the start of
            # the kernel instead of gating the first streaming exp
            warm_act = small.tile([P, 1], F32)
            nc.scalar.activation(
                out=warm_act[:], in_=nbias[:], func=mybir.ActivationFunctionType.Exp
            )

            # pair-fold matrix W[p, f] = (p >> 1) == f, so W.T @ v gives
            # v[2f] + v[2f+1] on partitions 0..63
            ji = small.tile([P, RPC], I32)
            nc.gpsimd.iota(ji[:], pattern=[[1, RPC]], base=0, channel_multiplier=0)
            pi = small.tile([P, 1], I32)
            nc.gpsimd.iota(pi[:], pattern=[[1, 1]], base=0, channel_multiplier=1)
            nc.vector.tensor_scalar(
                out=pi[:], in0=pi[:], scalar1=1, scalar2=None,
                op0=mybir.AluOpType.arith_shift_right,
            )
            pf = small.tile([P, 1], F32)
            nc.vector.tensor_copy(pf[:], pi[:])
            wfold = small.tile([P, RPC], F32)
            nc.vector.tensor_scalar(
                out=wfold[:], in0=ji[:], scalar1=pf[:], scalar2=None,
                op0=mybir.AluOpType.is_equal,
            )

            # ---- target gather + margin math on [64, 1] tensors
            idx_sb = small.tile([RPC, 1], I32)
            nc.gpsimd.dma_start(out=idx_sb[:], in_=gidx_p[:])
            gc = small.tile([RPC, 1], F32)
            nc.gpsimd.indirect_dma_start(
                out=gc[:],
                out_offset=None,
                in_=cos_flat,
                in_offset=bass.IndirectOffsetOnAxis(ap=idx_sb[:], axis=0),
            )

            cg = small.tile([RPC, 1], F32)
            nc.vector.tensor_scalar(
                out=cg[:], in0=gc[:], scalar1=1.0 - EPS, scalar2=-1.0 + EPS,
                op0=mybir.AluOpType.min, op1=mybir.AluOpType.max,
            )
            # om = 1 - c^2  (via (c*c)*-1 + 1)
            om = small.tile([RPC, 1], F32)
            nc.vector.tensor_tensor(out=om[:], in0=cg[:], in1=cg[:], op=mybir.AluOpType.mult)
            nc.vector.tensor_scalar(
                out=om[:], in0=om[:], scalar1=-1.0, scalar2=1.0,
                op0=mybir.AluOpType.mult, op1=mybir.AluOpType.add,
            )
            # sine = exp(0.5 * ln(om)) — stays in the exp/ln table set
            sine = small.tile([RPC, 1], F32)
            nc.scalar.activation(out=sine[:], in_=om[:], func=mybir.ActivationFunctionType.Ln)
            nc.scalar.activation(
                out=sine[:], in_=sine[:], func=mybir.ActivationFunctionType.Exp, scale=0.5
            )
            # phi = c*COS_M - sine*SIN_M
            phi = small.tile([RPC, 1], F32)
            t1 = small.tile([RPC, 1], F32)
            nc.vector.tensor_scalar(
                out=t1[:], in0=sine[:], scalar1=SIN_M, scalar2=None, op0=mybir.AluOpType.mult
            )
            nc.vector.scalar_tensor_tensor(
                out=phi[:], in0=cg[:], scalar=COS_M, in1=t1[:],
                op0=mybir.AluOpType.mult, op1=mybir.AluOpType.subtract,
            )
            # phi = where(c > TH, phi, c - MM)
            gt = small.tile([RPC, 1], F32)
            nc.vector.tensor_scalar(
                out=gt[:], in0=cg[:], scalar1=TH, scalar2=None, op0=mybir.AluOpType.is_gt
            )
            cmm = small.tile([RPC, 1], F32)
            nc.vector.tensor_scalar(
                out=cmm[:], in0=cg[:], scalar1=MM, scalar2=None, op0=mybir.AluOpType.subtract
            )
            d = small.tile([RPC, 1], F32)
            nc.vector.tensor_tensor(out=d[:], in0=phi[:], in1=cmm[:], op=mybir.AluOpType.subtract)
            nc.vector.tensor_tensor(out=d[:], in0=d[:], in1=gt[:], op=mybir.AluOpType.mult)
            nc.vector.tensor_tensor(out=phi[:], in0=cmm[:], in1=d[:], op=mybir.AluOpType.add)

            # tS = S * phi (target logit); delta = e^(S*phi-64) - e^(S*c-64)
            tS = small.tile([RPC, 1], F32)
            nc.vector.tensor_scalar(
                out=tS[:], in0=phi[:], scalar1=S, scalar2=None, op0=mybir.AluOpType.mult
            )
            e_phi = small.tile([RPC, 1], F32)
            e_c = small.tile([RPC, 1], F32)
            nc.scalar.activation(
                out=e_phi[:], in_=phi[:], func=mybir.ActivationFunctionType.Exp,
                scale=S, bias=nbias[0:RPC, :],
            )
            nc.scalar.activation(
                out=e_c[:], in_=cg[:], func=mybir.ActivationFunctionType.Exp,
                scale=S, bias=nbias[0:RPC, :],
            )
            delta = small.tile([RPC, 1], F32)
            nc.vector.tensor_tensor(
                out=delta[:], in0=e_phi[:], in1=e_c[:], op=mybir.AluOpType.subtract
            )

            # ---- main streaming pass: exp + per-partition row-sum accum
            sums = small.tile([P, NCH], F32)
            off = 0
            for i, w in enumerate(CHUNKS):
                dt = data_pool.tile([P, FC_MAX], F32, tag="data")
                nc.sync.dma_start(out=dt[:, 0:w], in_=cos_r[:, off : off + w])
                ev = exp_pool.tile([P, FC_MAX], F32, tag="exp")
                nc.scalar.activation(
                    out=ev[:, 0:w], in_=dt[:, 0:w],
                    func=mybir.ActivationFunctionType.Exp,
                    scale=S, bias=nbias[:],
                    accum_out=sums[:, i : i + 1],
                )
                off += w

            # ---- fold halves, add margin delta, log, subtract target, reduce
            red = small.tile([P, 1], F32)
            nc.vector.tensor_reduce(
                out=red[:], in_=sums[:], axis=mybir.AxisListType.X, op=mybir.AluOpType.add
            )
            folded = psum.tile([RPC, 1], F32)
            nc.tensor.matmul(folded[:], wfold[:], red[:], start=True, stop=True)
            rowsum = small.tile([RPC, 1], F32)
            nc.vector.tensor_tensor(
                out=rowsum[:], in0=folded[:], in1=delta[:], op=mybir.AluOpType.add
            )
            logv = small.tile([RPC, 1], F32)
            nc.scalar.activation(
                out=logv[:], in_=rowsum[:], func=mybir.ActivationFunctionType.Ln
            )
            lvec = small.tile([RPC, 1], F32)
            nc.vector.tensor_tensor(
                out=lvec[:], in0=logv[:], in1=tS[:], op=mybir.AluOpType.subtract
            )
            ltot = small.tile([RPC, 1], F32)
            nc.gpsimd.partition_all_reduce(
                ltot[:], lvec[:], channels=RPC, reduce_op=bass_isa.ReduceOp.add
            )
            res = small.tile([1, 1], F32)
            nc.vector.tensor_copy(res[:], ltot[0:1, :])
            nc.sync.dma_start(out=out_p[:], in_=res[:])

    nc.finalize()
    return nc


_CACHE = {}


def _get_nc():
    if "nc" not in _CACHE:
        _CACHE["nc"] = build_nc()
    return _CACHE["nc"]


def make_in_maps(cosine: np.ndarray, labels: np.ndarray):
    labels = np.asarray(labels).astype(np.int64)
    rows = np.arange(RPC, dtype=np.int64)
    in_maps = []
    for m in range(NCORES):
        lo = m * RPC
        gidx = (rows * C + labels[lo : lo + RPC]).astype(np.int32).reshape(RPC, 1)
        in_maps.append({"cosine": cosine[lo : lo + RPC], "gidx": gidx})
    return in_maps


def kernel(cosine: np.ndarray, labels: np.ndarray, _trace: bool = False):
    nc = _get_nc()
    in_maps = make_in_maps(np.asarray(cosine, dtype=np.float32), labels)
    res = run_bass_kernel_spmd(
        nc, in_maps, core_ids=list(range(NCORES)), trace=_trace
    )
    parts = [np.asarray(r["out"], dtype=np.float64).reshape(()) for r in res.results]
    out = np.float32(SHIFT + sum(parts) / B)
    out = np.asarray(out, dtype=np.float32).reshape(())
    if _trace:
        return out, res
    return out
